# revision 18
# baseline (speedup 1.0000x reference)
"""Trainium2 Bass kernel for nn_CEBlock (topk_masking).

Strategy:
- Pure data parallelism: 1 sample per NeuronCore (B=8, 8 cores).
- Device computes per sample in feature-major ("transposed") layout:
  LN1 (stats precomputed on host from the raw input) -> qkv -> attention
  (scores^T per key-chunk, exp, unnormalized @v with a ones-column yielding
  softmax denominators for free) -> per-head-pair normalization -> proj ->
  residual -> LN2 (on-device stats) -> MLP on ALL 1008 tokens -> residual.
- The candidate-elimination ranking/sort/gather is done on host from device
  outputs (unnormalized exp-attention E^T and denominators d). LN2/MLP are
  per-token ops, so device applies them to all tokens and host gathers rows
  afterwards — mathematically identical to the reference's gather-then-MLP.
- Matmul dtypes: attention path float32r (full PE rate), MLP bf16, fp32
  accumulation in PSUM.
"""
import sys
for _p in ("/opt/trn_rl_repo", "/root/.axon_site/_ro/trn_rl_repo"):
    if _p not in sys.path:
        sys.path.insert(0, _p)

import numpy as np
import ml_dtypes

from concourse import bacc, mybir, tile
from concourse import bass_utils

# hardcoded problem constants (nn_CEBlock_17557826306354)
B = 8
DIM = 768
HEADS = 12
HD = 64
HID = 3072
LT = 432
LS = 576
NT = LT + LS           # 1008
KEEP = 404             # ceil(0.7*576)
EPS = 1e-5
SCALE = HD ** -0.5

FC = DIM // 128        # 6
TC = (NT + 127) // 128 # 8 (7x128 + 112)
MC1 = HID // 128       # 24
N0, N1 = 512, NT - 512

F32 = mybir.dt.float32
F32R = mybir.dt.float32r
BF16 = mybir.dt.bfloat16
AF = mybir.ActivationFunctionType
OP = mybir.AluOpType


def tsize(t):
    return 128 if t < TC - 1 else NT - 128 * (TC - 1)


def build_nc():
    nc = bacc.Bacc(None, target_bir_lowering=False)

    xT = nc.dram_tensor("xT", [DIM, NT], F32, kind="ExternalInput")
    mu1 = nc.dram_tensor("mu1", [1, NT], F32, kind="ExternalInput")
    rs1 = nc.dram_tensor("rs1", [1, NT], F32, kind="ExternalInput")
    qkv_wT = nc.dram_tensor("qkv_wT", [DIM, 3 * DIM], F32, kind="ExternalInput")
    proj_wT = nc.dram_tensor("proj_wT", [DIM, DIM], F32, kind="ExternalInput")
    fc1_wT = nc.dram_tensor("fc1_wT", [DIM, HID], BF16, kind="ExternalInput")
    fc2_wT = nc.dram_tensor("fc2_wT", [HID, DIM], BF16, kind="ExternalInput")
    n1w = nc.dram_tensor("n1w", [DIM, 1], F32, kind="ExternalInput")
    n1b = nc.dram_tensor("n1b", [DIM, 1], F32, kind="ExternalInput")
    n2w = nc.dram_tensor("n2w", [DIM, 1], F32, kind="ExternalInput")
    n2b = nc.dram_tensor("n2b", [DIM, 1], F32, kind="ExternalInput")
    fc1b = nc.dram_tensor("fc1b", [128, MC1], F32, kind="ExternalInput")
    projb = nc.dram_tensor("projb", [1, DIM], F32, kind="ExternalInput")
    fc2b = nc.dram_tensor("fc2b", [1, DIM], BF16, kind="ExternalInput")

    x2T = nc.dram_tensor("x2T", [DIM, NT], F32, kind="ExternalOutput")
    ace = nc.dram_tensor("ace", [HEADS, NT, LS], F32, kind="ExternalOutput")
    dall = nc.dram_tensor("dall", [HEADS, NT], F32, kind="ExternalOutput")

    with tile.TileContext(nc) as tc:
        with tc.tile_pool(name="const", bufs=1) as cpool, \
             tc.tile_pool(name="dram", bufs=1, space="DRAM") as dpool, \
             tc.tile_pool(name="pP", bufs=1) as pP:

            ones_f = cpool.tile([128, 12], F32, tag="ones_f")
            nc.vector.memset(ones_f[:], 1.0)
            ones_col = cpool.tile([128, 1], F32R, tag="ones_col")
            nc.vector.tensor_copy(ones_col[:], ones_f[:, 0:1])
            ones12 = cpool.tile([128, 12], F32R, tag="ones12")
            nc.vector.tensor_copy(ones12[:], ones_f[:])
            ones_rf = cpool.tile([1, NT], F32, tag="ones_rf")
            nc.vector.memset(ones_rf[:], 1.0)
            ones_row_r = cpool.tile([1, NT], F32R, tag="ones_row_r")
            nc.vector.tensor_copy(ones_row_r[:], ones_rf[:])
            ones_row_bf = cpool.tile([1, NT], BF16, tag="ones_row_bf")
            nc.vector.memset(ones_row_bf[:], 1.0)
            ones_r128 = cpool.tile([1, 128], F32R, tag="ones_r128")
            nc.vector.tensor_copy(ones_r128[:], ones_rf[:, 0:128])
            sel2f = cpool.tile([2, 128], F32, tag="sel2f")
            nc.vector.memset(sel2f[:], 0.0)
            nc.sync.dma_start(sel2f[0:1, 0:64], ones_rf[0:1, 0:64])
            nc.sync.dma_start(sel2f[1:2, 64:128], ones_rf[0:1, 0:64])


            nw_sb = {}
            for nm, t_ in (("n1w", n1w), ("n1b", n1b), ("n2w", n2w),
                           ("n2b", n2b)):
                nw_sb[nm] = [cpool.tile([128, 1], F32, tag=f"{nm}{c}",
                                        name=f"{nm}{c}") for c in range(FC)]
                for c in range(FC):
                    nc.sync.dma_start(nw_sb[nm][c][:],
                                      t_[128 * c:128 * (c + 1), :])
            fc1b_sb = cpool.tile([128, MC1], F32, tag="fc1b")
            nc.sync.dma_start(fc1b_sb[:], fc1b[:])
            projb_sb = cpool.tile([1, DIM], F32R, tag="projb")
            nc.sync.dma_start(projb_sb[:], projb[:].bitcast(F32R))
            fc2b_sb = cpool.tile([1, DIM], BF16, tag="fc2b")
            nc.sync.dma_start(fc2b_sb[:], fc2b[:])

            # ======== era Q: through proj ========
            with tc.tile_pool(name="pQ", bufs=1) as pQ:
                qkT = [pQ.tile([128, NT], F32R, tag=f"qkT{m}", name=f"qkT{m}")
                       for m in range(2 * FC)]
                v_sb = [pQ.tile([128, HEADS * 65], F32R, tag=f"v{t}",
                                name=f"v{t}") for t in range(TC)]
                attn_un = [pQ.tile([128, NT], F32, tag=f"aou{c}",
                                   name=f"aou{c}") for c in range(FC)]
                d_all = pP.tile([HEADS, NT], F32, tag="d_all")

                # ---- era R: LN1 (host stats) + qkv ----
                with tc.tile_pool(name="pR", bufs=1) as pR:
                    xnT = []
                    with tc.tile_pool(name="lnw", bufs=2) as lnw, \
                         tc.tile_pool(name="ln1ps", bufs=1,
                                      space="PSUM") as l1ps:
                        mu_r = lnw.tile([1, NT], F32, tag="mu_r", bufs=1)
                        nc.sync.dma_start(mu_r[:], mu1[:])
                        rs_r = lnw.tile([1, NT], F32, tag="rs_r", bufs=1)
                        nc.sync.dma_start(rs_r[:], rs1[:])
                        mu_b = l1ps.tile([128, NT], F32, tag="mu_b")
                        rs_b = l1ps.tile([128, NT], F32, tag="rs_b")
                        for lo, n in ((0, N0), (N0, N1)):
                            nc.tensor.matmul(mu_b[:, lo:lo + n],
                                             ones_rf[:, 0:128],
                                             mu_r[:, lo:lo + n],
                                             start=True, stop=True)
                            nc.tensor.matmul(rs_b[:, lo:lo + n],
                                             ones_rf[:, 0:128],
                                             rs_r[:, lo:lo + n],
                                             start=True, stop=True)
                        for c in range(FC):
                            xc = lnw.tile([128, NT], F32, tag="xc", bufs=2,
                                          name="xc")
                            nc.sync.dma_start(xc[:],
                                              xT[128 * c:128 * (c + 1), :])
                            t1 = lnw.tile([128, NT], F32, tag="t1", bufs=2,
                                          name="t1")
                            nc.vector.tensor_tensor(t1[:], xc[:], mu_b[:],
                                                    op=OP.subtract)
                            t2 = lnw.tile([128, NT], F32, tag="t2", bufs=2,
                                          name="t2")
                            nc.vector.tensor_tensor(t2[:], t1[:], rs_b[:],
                                                    op=OP.mult)
                            o = pR.tile([128, NT], F32R, tag=f"xn{c}",
                                        name=f"xn{c}")
                            nc.vector.tensor_scalar(o[:], t2[:],
                                                    nw_sb["n1w"][c][:],
                                                    nw_sb["n1b"][c][:],
                                                    op0=OP.mult, op1=OP.add)
                            xnT.append(o)

                    with tc.tile_pool(name="wsl", bufs=1) as wsl, \
                         tc.tile_pool(name="qkps", bufs=3, space="PSUM") as qps:
                        wqk = [wsl.tile([128, 2 * DIM], F32R, tag=f"wqk{kc}",
                                        name=f"wqk{kc}") for kc in range(FC)]
                        for kc in range(FC):
                            nc.sync.dma_start(
                                wqk[kc][:],
                                qkv_wT[128 * kc:128 * (kc + 1), 0:2 * DIM]
                                .bitcast(F32R))
                        for m in range(2 * FC):
                            ps = qps.tile([128, NT], F32, tag="qk")
                            for kc in range(FC):
                                for lo, n in ((0, N0), (N0, N1)):
                                    nc.tensor.matmul(
                                        ps[:, lo:lo + n],
                                        wqk[kc][:, 128 * m:128 * (m + 1)],
                                        xnT[kc][:, lo:lo + n],
                                        start=(kc == 0), stop=(kc == FC - 1))
                            nc.scalar.copy(qkT[m][:], ps[:])
                    with tc.tile_pool(name="wv", bufs=1) as wv, \
                         tc.tile_pool(name="vps", bufs=3, space="PSUM") as vps:
                        wvt = [wv.tile([128, DIM], F32R, tag=f"wv{kc}",
                                       name=f"wv{kc}") for kc in range(FC)]
                        for kc in range(FC):
                            nc.sync.dma_start(
                                wvt[kc][:],
                                qkv_wT[128 * kc:128 * (kc + 1), 2 * DIM:]
                                .bitcast(F32R))
                        for t in range(TC):
                            pt = tsize(t)
                            ps = vps.tile([128, DIM], F32, tag="v")
                            for kc in range(FC):
                                for lo, n in ((0, 512), (512, 256)):
                                    nc.tensor.matmul(
                                        ps[:pt, lo:lo + n],
                                        xnT[kc][:, 128 * t:128 * t + pt],
                                        wvt[kc][:, lo:lo + n],
                                        start=(kc == 0), stop=(kc == FC - 1))
                            nc.vector.tensor_copy(
                                v_sb[t][:pt, :]
                                .rearrange("p (h e) -> p h e", e=65)[:, :, 64:65],
                                ones12[:pt, :])
                            nc.vector.tensor_copy(
                                v_sb[t][:pt, :]
                                .rearrange("p (h e) -> p h e", e=65)[:, :, 0:64],
                                ps[:pt, :].rearrange("p (h e) -> p h e", e=64))

                # proj weights: load early, overlapping attention
                pw_cm = tc.tile_pool(name="pw", bufs=1)
                pw = pw_cm.__enter__()
                pw_sb = [pw.tile([128, DIM], F32R, tag=f"pw{kc}",
                                 name=f"pw{kc}") for kc in range(FC)]
                for kc in range(FC):
                    nc.sync.dma_start(pw_sb[kc][:],
                                      proj_wT[128 * kc:128 * (kc + 1), :]
                                      .bitcast(F32R))

                # ---- attention; normalize per head-pair as d becomes ready
                with tc.tile_pool(name="att", bufs=3) as ap_, \
                     tc.tile_pool(name="stg", bufs=2) as stp, \
                     tc.tile_pool(name="dn", bufs=2) as dn, \
                     tc.tile_pool(name="attps", bufs=2, space="PSUM") as aps:
                    rd_rows = []
                    for h in range(HEADS):
                        q_h = qkT[(64 * h) // 128][(64 * h) % 128:
                                                   (64 * h) % 128 + 64, :]
                        kk = DIM + 64 * h
                        k_h = qkT[kk // 128][kk % 128:kk % 128 + 64, :]
                        ps_o = aps.tile([65, NT], F32, tag="o")
                        for t in range(TC):
                            pt = tsize(t)
                            ps_s = aps.tile([128, NT], F32, tag="s")
                            for lo, n in ((0, N0), (N0, N1)):
                                nc.tensor.matmul(ps_s[:pt, lo:lo + n],
                                                 k_h[:, 128 * t:128 * t + pt],
                                                 q_h[:, lo:lo + n],
                                                 start=True, stop=True)
                            e_t = ap_.tile([128, NT], F32R, tag="e")
                            nc.scalar.activation(e_t[:pt, :], ps_s[:pt, :],
                                                 AF.Exp, scale=SCALE)
                            nc.sync.dma_start(ace[h, 128 * t:128 * t + pt, :],
                                              e_t[:pt, LT:NT].bitcast(F32))
                            for lo, n in ((0, N0), (N0, N1)):
                                nc.tensor.matmul(
                                    ps_o[:, lo:lo + n],
                                    v_sb[t][:pt, 65 * h:65 * (h + 1)],
                                    e_t[:pt, lo:lo + n],
                                    start=(t == 0), stop=(t == TC - 1))
                        stage = stp.tile([65, NT], F32, tag="stage")
                        nc.vector.tensor_copy(stage[:], ps_o[:])
                        c0, r0 = (64 * h) // 128, (64 * h) % 128
                        nc.sync.dma_start(attn_un[c0][r0:r0 + 64, :],
                                          stage[0:64, :])
                        nc.sync.dma_start(d_all[h:h + 1, :], stage[64:65, :])

                        if h % 2 == 1:
                            c = h // 2
                            d_pr = dpool.tile([2, NT], F32, tag=f"d_pr{c}",
                                              name=f"d_pr{c}")
                            nc.sync.dma_start(d_pr[:], d_all[h - 1:h + 1, :])
                            d_sm = dn.tile([126, 16], F32, tag="d_sm")
                            nc.sync.dma_start(
                                d_sm[:],
                                d_pr[:].rearrange("h (a b) -> (h a) b", a=63))
                            rd_sm = dn.tile([126, 16], F32, tag="rd_sm")
                            nc.vector.reciprocal(rd_sm[:], d_sm[:])
                            rd_pr = dpool.tile([2, NT], F32, tag=f"rd_pr{c}",
                                               name=f"rd_pr{c}")
                            nc.sync.dma_start(
                                rd_pr[:].rearrange("h (a b) -> (h a) b", a=63),
                                rd_sm[:])
                            rd2 = dn.tile([2, NT], F32, tag="rd2",
                                          name=f"rd2_{c}", bufs=2)
                            nc.sync.dma_start(rd2[:], rd_pr[:])
                            rd_rows.append(rd2)

                            def norm_chunk(cc):
                                rdc = rd_rows[cc]
                                rb = aps.tile([128, NT], F32, tag="s",
                                              name="rb")
                                for lo, n in ((0, N0), (N0, N1)):
                                    nc.tensor.matmul(rb[:, lo:lo + n],
                                                     sel2f[:],
                                                     rdc[:, lo:lo + n],
                                                     start=True, stop=True)
                                nc.vector.tensor_tensor(
                                    attn_un[cc][:].bitcast(F32R),
                                    attn_un[cc][:], rb[:], op=OP.mult)

                            # normalize one pair behind so the d-recip DMA
                            # chain is hidden under the next pair's compute
                            if c >= 1:
                                norm_chunk(c - 1)
                    norm_chunk(FC - 1)
                nc.sync.dma_start(dall[:], d_all[:])

                # ---- proj + residual -> x1T (f32r) ----
                x1T = [pP.tile([128, NT], F32R, tag=f"x1T{c}", name=f"x1T{c}")
                       for c in range(FC)]
                with tc.tile_pool(name="pps", bufs=2, space="PSUM") as pps:
                    for m in range(FC):
                        ps = pps.tile([128, NT], F32, tag="y")
                        for kc in range(FC):
                            for lo, n in ((0, N0), (N0, N1)):
                                nc.tensor.matmul(
                                    ps[:, lo:lo + n],
                                    pw_sb[kc][:, 128 * m:128 * (m + 1)],
                                    attn_un[kc][:, lo:lo + n].bitcast(F32R),
                                    start=(kc == 0), stop=False)
                        for lo, n in ((0, N0), (N0, N1)):
                            nc.tensor.matmul(
                                ps[:, lo:lo + n],
                                projb_sb[:, 128 * m:128 * (m + 1)],
                                ones_row_r[:, lo:lo + n],
                                start=False, stop=True)
                        xc = pw.tile([128, NT], F32, tag="xc2", bufs=2,
                                     name="xc2")
                        nc.sync.dma_start(xc[:], xT[128 * m:128 * (m + 1), :])
                        nc.vector.tensor_tensor(x1T[m][:], ps[:], xc[:],
                                                op=OP.add)
                pw_cm.__exit__(None, None, None)

            # ======== LN2 (device stats) + MLP on all tokens ========
            mlp_cm = tc.tile_pool(name="pH", bufs=1)
            pH = mlp_cm.__enter__()
            with tc.tile_pool(name="ln2", bufs=2) as lp, \
                 tc.tile_pool(name="ln2s", bufs=1) as ls, \
                 tc.tile_pool(name="ln2ps", bufs=1, space="PSUM") as lps:
                ps_sum = lps.tile([1, NT], F32, tag="ps_sum")
                ps_sq = lps.tile([1, NT], F32, tag="ps_sq")
                for c in range(FC):
                    sq = lp.tile([128, NT], F32R, tag="sq")
                    nc.scalar.activation(sq[:], x1T[c][:], AF.Square)
                    for lo, n in ((0, N0), (N0, N1)):
                        nc.tensor.matmul(ps_sum[:, lo:lo + n], ones_col[:],
                                         x1T[c][:, lo:lo + n],
                                         start=(c == 0), stop=(c == FC - 1))
                        nc.tensor.matmul(ps_sq[:, lo:lo + n], ones_col[:],
                                         sq[:, lo:lo + n],
                                         start=(c == 0), stop=(c == FC - 1))
                mu = ls.tile([1, NT], F32, tag="mu")
                nc.vector.tensor_scalar(mu[:], ps_sum[:], 1.0 / DIM, None,
                                        op0=OP.mult)
                d_mu = dpool.tile([1, NT], F32, tag="d_mu2")
                nc.sync.dma_start(d_mu[:], mu[:])
                var = ls.tile([1, NT], F32, tag="var")
                musq = ls.tile([1, NT], F32, tag="musq")
                nc.vector.tensor_tensor(musq[:], mu[:], mu[:], op=OP.mult)
                nc.vector.tensor_scalar(var[:], ps_sq[:], 1.0 / DIM, EPS,
                                        op0=OP.mult, op1=OP.add)
                var2 = ls.tile([1, NT], F32, tag="mu", name="var2")
                nc.vector.tensor_tensor(var2[:], var[:], musq[:],
                                        op=OP.subtract)
                sig = ls.tile([1, NT], F32, tag="var", name="sig")
                nc.scalar.activation(sig[:], var2[:], AF.Sqrt)
                d_sig = dpool.tile([1, NT], F32, tag="d_sig2")
                nc.sync.dma_start(d_sig[:], sig[:])
                sig_sm = ls.tile([126, 8], F32, tag="sig_sm")
                nc.sync.dma_start(sig_sm[:],
                                  d_sig[:].rearrange("o (p f) -> (o p) f", f=8))
                rs_sm = ls.tile([126, 8], F32, tag="rs_sm")
                nc.vector.reciprocal(rs_sm[:], sig_sm[:])
                d_rs = dpool.tile([1, NT], F32, tag="d_rs2")
                nc.sync.dma_start(d_rs[:].rearrange("o (p f) -> (o p) f", f=8),
                                  rs_sm[:])
                mu_rr = ls.tile([1, NT], F32, tag="mu_rr")
                nc.sync.dma_start(mu_rr[:], d_mu[:])
                rs_rr = ls.tile([1, NT], F32, tag="rs_rr")
                nc.sync.dma_start(rs_rr[:], d_rs[:])
                mu_b2 = lps.tile([128, NT], F32, tag="mu_b2")
                rs_b2 = lps.tile([128, NT], F32, tag="rs_b2")
                for lo, n in ((0, N0), (N0, N1)):
                    nc.tensor.matmul(mu_b2[:, lo:lo + n],
                                     ones_rf[:, 0:128],
                                     mu_rr[:, lo:lo + n], start=True,
                                     stop=True)
                    nc.tensor.matmul(rs_b2[:, lo:lo + n],
                                     ones_rf[:, 0:128],
                                     rs_rr[:, lo:lo + n], start=True,
                                     stop=True)
                xn2T = []
                for c in range(FC):
                    t1 = lp.tile([128, NT], F32, tag="t1", bufs=1)
                    nc.vector.tensor_tensor(t1[:], x1T[c][:], mu_b2[:],
                                            op=OP.subtract)
                    t2 = lp.tile([128, NT], F32, tag="t2", bufs=1)
                    nc.vector.tensor_tensor(t2[:], t1[:], rs_b2[:],
                                            op=OP.mult)
                    o = pH.tile([128, NT], BF16, tag=f"xn2{c}",
                                name=f"xn2{c}")
                    nc.vector.tensor_scalar(o[:], t2[:], nw_sb["n2w"][c][:],
                                            nw_sb["n2b"][c][:], op0=OP.mult,
                                            op1=OP.add)
                    xn2T.append(o)

            h_sb = [pH.tile([128, NT], BF16, tag=f"h{m}", name=f"h{m}")
                    for m in range(MC1)]
            with tc.tile_pool(name="f1s", bufs=2) as f1s, \
                 tc.tile_pool(name="f2s", bufs=2) as f2s, \
                 tc.tile_pool(name="x2s", bufs=2) as x2s, \
                 tc.tile_pool(name="hps", bufs=2, space="PSUM") as hps, \
                 tc.tile_pool(name="fps", bufs=1, space="PSUM") as fps:
                fw = [f1s.tile([128, HID], BF16, tag=f"fw{kc}", bufs=1,
                               name=f"fw{kc}") for kc in range(FC)]
                for kc in range(FC):
                    nc.sync.dma_start(fw[kc][:],
                                      fc1_wT[128 * kc:128 * (kc + 1), :])
                for m in range(MC1):
                    ps = hps.tile([128, NT], F32, tag="h")
                    for kc in range(FC):
                        for lo, n in ((0, N0), (N0, N1)):
                            nc.tensor.matmul(ps[:, lo:lo + n],
                                             fw[kc][:, 128 * m:128 * (m + 1)],
                                             xn2T[kc][:, lo:lo + n],
                                             start=(kc == 0),
                                             stop=(kc == FC - 1))
                    nc.scalar.activation(h_sb[m][:], ps[:], AF.Gelu,
                                         bias=fc1b_sb[:, m:m + 1])

                # fc2 in 3 passes of 2 output chunks so it can start while
                # fc1 is still producing later h chunks
                f2w = [f2s.tile([128, DIM], BF16, tag=f"f2w{kc}", bufs=1,
                                name=f"f2w{kc}") for kc in range(MC1)]
                for kc in range(MC1):
                    nc.sync.dma_start(f2w[kc][:],
                                      fc2_wT[128 * kc:128 * (kc + 1), :])
                for half in range(3):
                    ms = (2 * half, 2 * half + 1)
                    pss = {}
                    for m in ms:
                        pss[m] = fps.tile([128, NT], F32, tag=f"f{m % 2}",
                                          name=f"f{m % 2}")
                    for kc in range(MC1):
                        for m in ms:
                            for lo, n in ((0, N0), (N0, N1)):
                                nc.tensor.matmul(
                                    pss[m][:, lo:lo + n],
                                    f2w[kc][:, 128 * m:128 * (m + 1)],
                                    h_sb[kc][:, lo:lo + n],
                                    start=(kc == 0), stop=False)
                    for m in ms:
                        for lo, n in ((0, N0), (N0, N1)):
                            nc.tensor.matmul(pss[m][:, lo:lo + n],
                                             fc2b_sb[:, 128 * m:128 * (m + 1)],
                                             ones_row_bf[:, lo:lo + n],
                                             start=False, stop=True)
                        x2c = x2s.tile([128, NT], F32, tag="x2c")
                        nc.vector.tensor_tensor(x2c[:], pss[m][:], x1T[m][:],
                                                op=OP.add)
                        nc.sync.dma_start(x2T[128 * m:128 * (m + 1), :],
                                          x2c[:])
            mlp_cm.__exit__(None, None, None)

    nc.finalize()
    return nc


_NC_CACHE = None
LAST_RESULT = None


def _get_nc():
    global _NC_CACHE
    if _NC_CACHE is None:
        _NC_CACHE = build_nc()
    return _NC_CACHE


def kernel(x, global_index_template, global_index_search, norm1_w, norm1_b,
           qkv_w, proj_w, proj_b, norm2_w, norm2_b, fc1_w, fc1_b, fc2_w,
           fc2_b):
    x = np.asarray(x, np.float32)
    shared = {
        "qkv_wT": np.ascontiguousarray(np.asarray(qkv_w, np.float32).T),
        "proj_wT": np.ascontiguousarray(np.asarray(proj_w, np.float32).T),
        "fc1_wT": np.ascontiguousarray(
            np.asarray(fc1_w).T.astype(ml_dtypes.bfloat16)),
        "fc2_wT": np.ascontiguousarray(
            np.asarray(fc2_w).T.astype(ml_dtypes.bfloat16)),
        "n1w": np.ascontiguousarray(np.asarray(norm1_w, np.float32)[:, None]),
        "n1b": np.ascontiguousarray(np.asarray(norm1_b, np.float32)[:, None]),
        "n2w": np.ascontiguousarray(np.asarray(norm2_w, np.float32)[:, None]),
        "n2b": np.ascontiguousarray(np.asarray(norm2_b, np.float32)[:, None]),
        "fc1b": np.ascontiguousarray(
            np.asarray(fc1_b, np.float32).reshape(MC1, 128).T),
        "projb": np.ascontiguousarray(np.asarray(proj_b, np.float32)[None, :]),
        "fc2b": np.ascontiguousarray(
            np.asarray(fc2_b).astype(ml_dtypes.bfloat16)[None, :]),
    }
    in_maps = []
    for b in range(B):
        m = dict(shared)
        xb = x[b]
        m["xT"] = np.ascontiguousarray(xb.T)
        mu = xb.mean(axis=1)
        var = ((xb - mu[:, None]) ** 2).mean(axis=1)
        m["mu1"] = np.ascontiguousarray(mu[None, :].astype(np.float32))
        m["rs1"] = np.ascontiguousarray(
            (1.0 / np.sqrt(var + EPS))[None, :].astype(np.float32))
        in_maps.append(m)

    nc = _get_nc()
    res = bass_utils.run_bass_kernel_spmd(nc, in_maps, core_ids=list(range(B)))
    global LAST_RESULT
    LAST_RESULT = res

    gis = np.asarray(global_index_search)
    x_out = np.empty((B, LT + KEEP, DIM), np.float32)
    keep_index = np.empty((B, KEEP), gis.dtype)
    removed_index = np.empty((B, LS - KEEP), gis.dtype)
    attn_ce = np.empty((B, HEADS, LS, NT), np.float32)
    for b in range(B):
        r = res.results[b]
        d = r["dall"]                        # [H, NT]
        eT = r["ace"]                        # [H, NT(keys), LS(search q)]
        a_ce = eT.transpose(0, 2, 1) / d[:, LT:, None]
        attn_ce[b] = a_ce
        attn_t = np.linalg.norm(a_ce.mean(axis=0), axis=1)
        order = np.argsort(-attn_t, kind="stable")
        topk = order[:KEEP]
        keep_index[b] = gis[b][topk]
        removed_index[b] = gis[b][order[KEEP:]]
        x2 = r["x2T"].T
        x_out[b, :LT] = x2[:LT]
        x_out[b, LT:] = x2[LT:][topk]

    return (x_out, np.asarray(global_index_template), keep_index,
            removed_index, attn_ce)


# revision 21
# speedup vs baseline: 1.0563x; 1.0563x over previous
"""Trainium2 Bass kernel for nn_CEBlock (topk_masking).

Strategy:
- Pure data parallelism: 1 sample per NeuronCore (B=8, 8 cores).
- Device computes per sample in feature-major ("transposed") layout:
  LN1 (stats precomputed on host from the raw input) -> qkv -> attention
  (scores^T per key-chunk, exp, unnormalized @v with a ones-column yielding
  softmax denominators for free) -> per-head-pair normalization -> proj ->
  residual -> LN2 (on-device stats) -> MLP on ALL 1008 tokens -> residual.
- The candidate-elimination ranking/sort/gather is done on host from device
  outputs (unnormalized exp-attention E^T and denominators d). LN2/MLP are
  per-token ops, so device applies them to all tokens and host gathers rows
  afterwards — mathematically identical to the reference's gather-then-MLP.
- Matmul dtypes: attention path float32r (full PE rate), MLP bf16, fp32
  accumulation in PSUM.
"""
import sys
for _p in ("/opt/trn_rl_repo", "/root/.axon_site/_ro/trn_rl_repo"):
    if _p not in sys.path:
        sys.path.insert(0, _p)

import numpy as np
import ml_dtypes

from concourse import bacc, mybir, tile
from concourse import bass_utils

# hardcoded problem constants (nn_CEBlock_17557826306354)
B = 8
DIM = 768
HEADS = 12
HD = 64
HID = 3072
LT = 432
LS = 576
NT = LT + LS           # 1008
KEEP = 404             # ceil(0.7*576)
EPS = 1e-5
SCALE = HD ** -0.5

FC = DIM // 128        # 6
TC = (NT + 127) // 128 # 8 (7x128 + 112)
MC1 = HID // 128       # 24
N0, N1 = 512, NT - 512

F32 = mybir.dt.float32
F32R = mybir.dt.float32r
BF16 = mybir.dt.bfloat16
AF = mybir.ActivationFunctionType
OP = mybir.AluOpType


def tsize(t):
    return 128 if t < TC - 1 else NT - 128 * (TC - 1)


def build_nc():
    nc = bacc.Bacc(None, target_bir_lowering=False)

    xT = nc.dram_tensor("xT", [DIM, NT], F32, kind="ExternalInput")
    mu1 = nc.dram_tensor("mu1", [1, NT], F32, kind="ExternalInput")
    rs1 = nc.dram_tensor("rs1", [1, NT], F32, kind="ExternalInput")
    qkv_wT = nc.dram_tensor("qkv_wT", [DIM, 3 * DIM], F32, kind="ExternalInput")
    proj_wT = nc.dram_tensor("proj_wT", [DIM, DIM], F32, kind="ExternalInput")
    fc1_wT = nc.dram_tensor("fc1_wT", [DIM, HID], BF16, kind="ExternalInput")
    fc2_wT = nc.dram_tensor("fc2_wT", [HID, DIM], BF16, kind="ExternalInput")
    n1w = nc.dram_tensor("n1w", [DIM, 1], F32, kind="ExternalInput")
    n1b = nc.dram_tensor("n1b", [DIM, 1], F32, kind="ExternalInput")
    n2w = nc.dram_tensor("n2w", [DIM, 1], F32, kind="ExternalInput")
    n2b = nc.dram_tensor("n2b", [DIM, 1], F32, kind="ExternalInput")
    fc1b = nc.dram_tensor("fc1b", [128, MC1], F32, kind="ExternalInput")
    projb = nc.dram_tensor("projb", [1, DIM], F32, kind="ExternalInput")
    fc2b = nc.dram_tensor("fc2b", [1, DIM], BF16, kind="ExternalInput")

    x2T = nc.dram_tensor("x2T", [DIM, NT], F32, kind="ExternalOutput")
    ace = nc.dram_tensor("ace", [HEADS, NT, LS], F32, kind="ExternalOutput")
    dall = nc.dram_tensor("dall", [HEADS, NT], F32, kind="ExternalOutput")

    with tile.TileContext(nc) as tc:
        with tc.tile_pool(name="const", bufs=1) as cpool, \
             tc.tile_pool(name="dram", bufs=1, space="DRAM") as dpool, \
             tc.tile_pool(name="pP", bufs=1) as pP:

            ones_f = cpool.tile([128, 12], F32, tag="ones_f")
            nc.vector.memset(ones_f[:], 1.0)
            ones_col = cpool.tile([128, 1], F32R, tag="ones_col")
            nc.vector.tensor_copy(ones_col[:], ones_f[:, 0:1])
            ones12 = cpool.tile([128, 12], F32R, tag="ones12")
            nc.vector.tensor_copy(ones12[:], ones_f[:])
            ones_rf = cpool.tile([1, NT], F32, tag="ones_rf")
            nc.vector.memset(ones_rf[:], 1.0)
            ones_row_r = cpool.tile([1, NT], F32R, tag="ones_row_r")
            nc.vector.tensor_copy(ones_row_r[:], ones_rf[:])
            ones_row_bf = cpool.tile([1, NT], BF16, tag="ones_row_bf")
            nc.vector.memset(ones_row_bf[:], 1.0)
            ones_r128 = cpool.tile([1, 128], F32R, tag="ones_r128")
            nc.vector.tensor_copy(ones_r128[:], ones_rf[:, 0:128])
            sel2f = cpool.tile([2, 128], F32, tag="sel2f")
            nc.vector.memset(sel2f[:], 0.0)
            nc.sync.dma_start(sel2f[0:1, 0:64], ones_rf[0:1, 0:64])
            nc.sync.dma_start(sel2f[1:2, 64:128], ones_rf[0:1, 0:64])


            nw_sb = {}
            for nm, t_ in (("n1w", n1w), ("n1b", n1b), ("n2w", n2w),
                           ("n2b", n2b)):
                nw_sb[nm] = [cpool.tile([128, 1], F32, tag=f"{nm}{c}",
                                        name=f"{nm}{c}") for c in range(FC)]
                for c in range(FC):
                    nc.sync.dma_start(nw_sb[nm][c][:],
                                      t_[128 * c:128 * (c + 1), :])
            fc1b_sb = cpool.tile([128, MC1], F32, tag="fc1b")
            nc.sync.dma_start(fc1b_sb[:], fc1b[:])
            projb_sb = cpool.tile([1, DIM], F32R, tag="projb")
            nc.sync.dma_start(projb_sb[:], projb[:].bitcast(F32R))
            fc2b_sb = cpool.tile([1, DIM], BF16, tag="fc2b")
            nc.sync.dma_start(fc2b_sb[:], fc2b[:])

            # ======== era Q: through proj ========
            with tc.tile_pool(name="pQ", bufs=1) as pQ:
                qkT = [pQ.tile([128, NT], F32R, tag=f"qkT{m}", name=f"qkT{m}")
                       for m in range(2 * FC)]
                v_sb = [pQ.tile([128, HEADS * 65], F32R, tag=f"v{t}",
                                name=f"v{t}") for t in range(TC)]
                attn_un = [pQ.tile([128, NT], F32, tag=f"aou{c}",
                                   name=f"aou{c}") for c in range(FC)]
                d_all = pP.tile([HEADS, NT], F32, tag="d_all")

                # ---- era R: LN1 (host stats) + qkv ----
                with tc.tile_pool(name="pR", bufs=1) as pR:
                    xnT = []
                    with tc.tile_pool(name="lnw", bufs=2) as lnw, \
                         tc.tile_pool(name="ln1ps", bufs=1,
                                      space="PSUM") as l1ps:
                        mu_r = lnw.tile([1, NT], F32, tag="mu_r", bufs=1)
                        nc.sync.dma_start(mu_r[:], mu1[:])
                        rs_r = lnw.tile([1, NT], F32, tag="rs_r", bufs=1)
                        nc.sync.dma_start(rs_r[:], rs1[:])
                        mu_b = l1ps.tile([128, NT], F32, tag="mu_b")
                        rs_b = l1ps.tile([128, NT], F32, tag="rs_b")
                        for lo, n in ((0, N0), (N0, N1)):
                            nc.tensor.matmul(mu_b[:, lo:lo + n],
                                             ones_rf[:, 0:128],
                                             mu_r[:, lo:lo + n],
                                             start=True, stop=True)
                            nc.tensor.matmul(rs_b[:, lo:lo + n],
                                             ones_rf[:, 0:128],
                                             rs_r[:, lo:lo + n],
                                             start=True, stop=True)
                        for c in range(FC):
                            xc = lnw.tile([128, NT], F32, tag="xc", bufs=2,
                                          name="xc")
                            nc.sync.dma_start(xc[:],
                                              xT[128 * c:128 * (c + 1), :])
                            t1 = lnw.tile([128, NT], F32, tag="t1", bufs=2,
                                          name="t1")
                            nc.vector.tensor_tensor(t1[:], xc[:], mu_b[:],
                                                    op=OP.subtract)
                            t2 = lnw.tile([128, NT], F32, tag="t2", bufs=2,
                                          name="t2")
                            nc.vector.tensor_tensor(t2[:], t1[:], rs_b[:],
                                                    op=OP.mult)
                            o = pR.tile([128, NT], F32R, tag=f"xn{c}",
                                        name=f"xn{c}")
                            nc.vector.tensor_scalar(o[:], t2[:],
                                                    nw_sb["n1w"][c][:],
                                                    nw_sb["n1b"][c][:],
                                                    op0=OP.mult, op1=OP.add)
                            xnT.append(o)

                    with tc.tile_pool(name="wsl", bufs=1) as wsl, \
                         tc.tile_pool(name="qkps", bufs=3, space="PSUM") as qps:
                        wqk = [wsl.tile([128, 2 * DIM], F32R, tag=f"wqk{kc}",
                                        name=f"wqk{kc}") for kc in range(FC)]
                        for kc in range(FC):
                            nc.sync.dma_start(
                                wqk[kc][:],
                                qkv_wT[128 * kc:128 * (kc + 1), 0:2 * DIM]
                                .bitcast(F32R))
                        for m in range(2 * FC):
                            ps = qps.tile([128, NT], F32, tag="qk")
                            for kc in range(FC):
                                for lo, n in ((0, N0), (N0, N1)):
                                    nc.tensor.matmul(
                                        ps[:, lo:lo + n],
                                        wqk[kc][:, 128 * m:128 * (m + 1)],
                                        xnT[kc][:, lo:lo + n],
                                        start=(kc == 0), stop=(kc == FC - 1))
                            nc.scalar.copy(qkT[m][:], ps[:])
                    with tc.tile_pool(name="wv", bufs=1) as wv, \
                         tc.tile_pool(name="vps", bufs=3, space="PSUM") as vps:
                        wvt = [wv.tile([128, DIM], F32R, tag=f"wv{kc}",
                                       name=f"wv{kc}") for kc in range(FC)]
                        for kc in range(FC):
                            nc.sync.dma_start(
                                wvt[kc][:],
                                qkv_wT[128 * kc:128 * (kc + 1), 2 * DIM:]
                                .bitcast(F32R))
                        for t in range(TC):
                            pt = tsize(t)
                            ps = vps.tile([128, DIM], F32, tag="v")
                            for kc in range(FC):
                                for lo, n in ((0, 512), (512, 256)):
                                    nc.tensor.matmul(
                                        ps[:pt, lo:lo + n],
                                        xnT[kc][:, 128 * t:128 * t + pt],
                                        wvt[kc][:, lo:lo + n],
                                        start=(kc == 0), stop=(kc == FC - 1))
                            nc.vector.tensor_copy(
                                v_sb[t][:pt, :]
                                .rearrange("p (h e) -> p h e", e=65)[:, :, 64:65],
                                ones12[:pt, :])
                            nc.vector.tensor_copy(
                                v_sb[t][:pt, :]
                                .rearrange("p (h e) -> p h e", e=65)[:, :, 0:64],
                                ps[:pt, :].rearrange("p (h e) -> p h e", e=64))

                # proj weights: load early, overlapping attention
                pw_cm = tc.tile_pool(name="pw", bufs=1)
                pw = pw_cm.__enter__()
                pw_sb = [pw.tile([128, DIM], F32R, tag=f"pw{kc}",
                                 name=f"pw{kc}") for kc in range(FC)]
                for kc in range(FC):
                    nc.sync.dma_start(pw_sb[kc][:],
                                      proj_wT[128 * kc:128 * (kc + 1), :]
                                      .bitcast(F32R))

                # ---- attention; normalize per head-pair as d becomes ready
                with tc.tile_pool(name="att", bufs=3) as ap_, \
                     tc.tile_pool(name="stg", bufs=2) as stp, \
                     tc.tile_pool(name="dn", bufs=2) as dn, \
                     tc.tile_pool(name="attps", bufs=2, space="PSUM") as aps:
                    rd_rows = []
                    for c in range(FC):
                        h0, h1 = 2 * c, 2 * c + 1
                        # head pair shares qkT chunk c (rows 0:64 / 64:128):
                        # score matmuls land on distinct PE row-groups and
                        # run concurrently in the array
                        q0 = qkT[c][0:64, :]
                        q1 = qkT[c][64:128, :]
                        k0 = qkT[FC + c][0:64, :]
                        k1 = qkT[FC + c][64:128, :]
                        ps_o0 = aps.tile([65, NT], F32, tag="o0", bufs=1)
                        ps_o1 = aps.tile([65, NT], F32, tag="o1", bufs=1)
                        for t in range(TC):
                            pt = tsize(t)
                            tsl = slice(128 * t, 128 * t + pt)
                            ps_s0 = aps.tile([128, NT], F32, tag="s",
                                             bufs=1, name="ps_s0")
                            ps_s1 = aps.tile([128, NT], F32, tag="s1",
                                             bufs=1, name="ps_s1")
                            for lo, n in ((0, N0), (N0, N1)):
                                nc.tensor.matmul(ps_s0[:pt, lo:lo + n],
                                                 k0[:, tsl], q0[:, lo:lo + n],
                                                 start=True, stop=True)
                                nc.tensor.matmul(ps_s1[:pt, lo:lo + n],
                                                 k1[:, tsl], q1[:, lo:lo + n],
                                                 start=True, stop=True)
                            e_0 = ap_.tile([128, NT], F32R, tag="e0",
                                           bufs=2, name="e_0")
                            nc.scalar.activation(e_0[:pt, :], ps_s0[:pt, :],
                                                 AF.Exp, scale=SCALE)
                            e_1 = ap_.tile([128, NT], F32R, tag="e1",
                                           bufs=2, name="e_1")
                            nc.scalar.activation(e_1[:pt, :], ps_s1[:pt, :],
                                                 AF.Exp, scale=SCALE)
                            nc.sync.dma_start(ace[h0, tsl, :],
                                              e_0[:pt, LT:NT].bitcast(F32))
                            nc.sync.dma_start(ace[h1, tsl, :],
                                              e_1[:pt, LT:NT].bitcast(F32))
                            for lo, n in ((0, N0), (N0, N1)):
                                nc.tensor.matmul(
                                    ps_o0[:, lo:lo + n],
                                    v_sb[t][:pt, 65 * h0:65 * (h0 + 1)],
                                    e_0[:pt, lo:lo + n],
                                    start=(t == 0), stop=(t == TC - 1))
                                nc.tensor.matmul(
                                    ps_o1[:, lo:lo + n],
                                    v_sb[t][:pt, 65 * h1:65 * (h1 + 1)],
                                    e_1[:pt, lo:lo + n],
                                    start=(t == 0), stop=(t == TC - 1))
                        for h, ps_o in ((h0, ps_o0), (h1, ps_o1)):
                            stage = stp.tile([65, NT], F32, tag="stage")
                            nc.vector.tensor_copy(stage[:], ps_o[:])
                            c0, r0 = (64 * h) // 128, (64 * h) % 128
                            nc.sync.dma_start(attn_un[c0][r0:r0 + 64, :],
                                              stage[0:64, :])
                            nc.sync.dma_start(d_all[h:h + 1, :],
                                              stage[64:65, :])

                        if True:
                            h = h1
                            d_pr = dpool.tile([2, NT], F32, tag=f"d_pr{c}",
                                              name=f"d_pr{c}")
                            nc.sync.dma_start(d_pr[:], d_all[h - 1:h + 1, :])
                            d_sm = dn.tile([126, 16], F32, tag="d_sm")
                            nc.sync.dma_start(
                                d_sm[:],
                                d_pr[:].rearrange("h (a b) -> (h a) b", a=63))
                            rd_sm = dn.tile([126, 16], F32, tag="rd_sm")
                            nc.vector.reciprocal(rd_sm[:], d_sm[:])
                            rd_pr = dpool.tile([2, NT], F32, tag=f"rd_pr{c}",
                                               name=f"rd_pr{c}")
                            nc.sync.dma_start(
                                rd_pr[:].rearrange("h (a b) -> (h a) b", a=63),
                                rd_sm[:])
                            rd2 = dn.tile([2, NT], F32, tag="rd2",
                                          name=f"rd2_{c}", bufs=2)
                            nc.sync.dma_start(rd2[:], rd_pr[:])
                            rd_rows.append(rd2)

                            def norm_chunk(cc):
                                rdc = rd_rows[cc]
                                rb = aps.tile([128, NT], F32, tag="s",
                                              bufs=1, name="rb")
                                for lo, n in ((0, N0), (N0, N1)):
                                    nc.tensor.matmul(rb[:, lo:lo + n],
                                                     sel2f[:],
                                                     rdc[:, lo:lo + n],
                                                     start=True, stop=True)
                                nc.vector.tensor_tensor(
                                    attn_un[cc][:].bitcast(F32R),
                                    attn_un[cc][:], rb[:], op=OP.mult)

                            # normalize one pair behind so the d-recip DMA
                            # chain is hidden under the next pair's compute
                            if c >= 1:
                                norm_chunk(c - 1)
                    norm_chunk(FC - 1)
                nc.sync.dma_start(dall[:], d_all[:])

                # ---- proj + residual -> x1T (f32r) ----
                x1T = [pP.tile([128, NT], F32R, tag=f"x1T{c}", name=f"x1T{c}")
                       for c in range(FC)]
                with tc.tile_pool(name="pps", bufs=2, space="PSUM") as pps:
                    for m in range(FC):
                        ps = pps.tile([128, NT], F32, tag="y")
                        for kc in range(FC):
                            for lo, n in ((0, N0), (N0, N1)):
                                nc.tensor.matmul(
                                    ps[:, lo:lo + n],
                                    pw_sb[kc][:, 128 * m:128 * (m + 1)],
                                    attn_un[kc][:, lo:lo + n].bitcast(F32R),
                                    start=(kc == 0), stop=False)
                        for lo, n in ((0, N0), (N0, N1)):
                            nc.tensor.matmul(
                                ps[:, lo:lo + n],
                                projb_sb[:, 128 * m:128 * (m + 1)],
                                ones_row_r[:, lo:lo + n],
                                start=False, stop=True)
                        xc = pw.tile([128, NT], F32, tag="xc2", bufs=2,
                                     name="xc2")
                        nc.sync.dma_start(xc[:], xT[128 * m:128 * (m + 1), :])
                        nc.vector.tensor_tensor(x1T[m][:], ps[:], xc[:],
                                                op=OP.add)
                pw_cm.__exit__(None, None, None)

            # ======== LN2 (device stats) + MLP on all tokens ========
            mlp_cm = tc.tile_pool(name="pH", bufs=1)
            pH = mlp_cm.__enter__()
            with tc.tile_pool(name="ln2", bufs=2) as lp, \
                 tc.tile_pool(name="ln2s", bufs=1) as ls, \
                 tc.tile_pool(name="ln2ps", bufs=1, space="PSUM") as lps:
                ps_sum = lps.tile([1, NT], F32, tag="ps_sum")
                ps_sq = lps.tile([1, NT], F32, tag="ps_sq")
                for c in range(FC):
                    sq = lp.tile([128, NT], F32R, tag="sq")
                    nc.scalar.activation(sq[:], x1T[c][:], AF.Square)
                    for lo, n in ((0, N0), (N0, N1)):
                        nc.tensor.matmul(ps_sum[:, lo:lo + n], ones_col[:],
                                         x1T[c][:, lo:lo + n],
                                         start=(c == 0), stop=(c == FC - 1))
                        nc.tensor.matmul(ps_sq[:, lo:lo + n], ones_col[:],
                                         sq[:, lo:lo + n],
                                         start=(c == 0), stop=(c == FC - 1))
                mu = ls.tile([1, NT], F32, tag="mu")
                nc.vector.tensor_scalar(mu[:], ps_sum[:], 1.0 / DIM, None,
                                        op0=OP.mult)
                d_mu = dpool.tile([1, NT], F32, tag="d_mu2")
                nc.sync.dma_start(d_mu[:], mu[:])
                var = ls.tile([1, NT], F32, tag="var")
                musq = ls.tile([1, NT], F32, tag="musq")
                nc.vector.tensor_tensor(musq[:], mu[:], mu[:], op=OP.mult)
                nc.vector.tensor_scalar(var[:], ps_sq[:], 1.0 / DIM, EPS,
                                        op0=OP.mult, op1=OP.add)
                var2 = ls.tile([1, NT], F32, tag="mu", name="var2")
                nc.vector.tensor_tensor(var2[:], var[:], musq[:],
                                        op=OP.subtract)
                sig = ls.tile([1, NT], F32, tag="var", name="sig")
                nc.scalar.activation(sig[:], var2[:], AF.Sqrt)
                d_sig = dpool.tile([1, NT], F32, tag="d_sig2")
                nc.sync.dma_start(d_sig[:], sig[:])
                sig_sm = ls.tile([126, 8], F32, tag="sig_sm")
                nc.sync.dma_start(sig_sm[:],
                                  d_sig[:].rearrange("o (p f) -> (o p) f", f=8))
                rs_sm = ls.tile([126, 8], F32, tag="rs_sm")
                nc.vector.reciprocal(rs_sm[:], sig_sm[:])
                d_rs = dpool.tile([1, NT], F32, tag="d_rs2")
                nc.sync.dma_start(d_rs[:].rearrange("o (p f) -> (o p) f", f=8),
                                  rs_sm[:])
                mu_rr = ls.tile([1, NT], F32, tag="mu_rr")
                nc.sync.dma_start(mu_rr[:], d_mu[:])
                rs_rr = ls.tile([1, NT], F32, tag="rs_rr")
                nc.sync.dma_start(rs_rr[:], d_rs[:])
                mu_b2 = lps.tile([128, NT], F32, tag="mu_b2")
                rs_b2 = lps.tile([128, NT], F32, tag="rs_b2")
                for lo, n in ((0, N0), (N0, N1)):
                    nc.tensor.matmul(mu_b2[:, lo:lo + n],
                                     ones_rf[:, 0:128],
                                     mu_rr[:, lo:lo + n], start=True,
                                     stop=True)
                    nc.tensor.matmul(rs_b2[:, lo:lo + n],
                                     ones_rf[:, 0:128],
                                     rs_rr[:, lo:lo + n], start=True,
                                     stop=True)
                xn2T = []
                for c in range(FC):
                    t1 = lp.tile([128, NT], F32, tag="t1", bufs=1)
                    nc.vector.tensor_tensor(t1[:], x1T[c][:], mu_b2[:],
                                            op=OP.subtract)
                    t2 = lp.tile([128, NT], F32, tag="t2", bufs=1)
                    nc.vector.tensor_tensor(t2[:], t1[:], rs_b2[:],
                                            op=OP.mult)
                    o = pH.tile([128, NT], BF16, tag=f"xn2{c}",
                                name=f"xn2{c}")
                    nc.vector.tensor_scalar(o[:], t2[:], nw_sb["n2w"][c][:],
                                            nw_sb["n2b"][c][:], op0=OP.mult,
                                            op1=OP.add)
                    xn2T.append(o)

            h_sb = [pH.tile([128, NT], BF16, tag=f"h{m}", name=f"h{m}")
                    for m in range(MC1)]
            with tc.tile_pool(name="f1s", bufs=2) as f1s, \
                 tc.tile_pool(name="f2s", bufs=2) as f2s, \
                 tc.tile_pool(name="x2s", bufs=2) as x2s, \
                 tc.tile_pool(name="hps", bufs=2, space="PSUM") as hps, \
                 tc.tile_pool(name="fps", bufs=1, space="PSUM") as fps:
                fw = [f1s.tile([128, HID], BF16, tag=f"fw{kc}", bufs=1,
                               name=f"fw{kc}") for kc in range(FC)]
                for kc in range(FC):
                    nc.sync.dma_start(fw[kc][:],
                                      fc1_wT[128 * kc:128 * (kc + 1), :])
                for m in range(MC1):
                    ps = hps.tile([128, NT], F32, tag="h")
                    for kc in range(FC):
                        for lo, n in ((0, N0), (N0, N1)):
                            nc.tensor.matmul(ps[:, lo:lo + n],
                                             fw[kc][:, 128 * m:128 * (m + 1)],
                                             xn2T[kc][:, lo:lo + n],
                                             start=(kc == 0),
                                             stop=(kc == FC - 1))
                    nc.scalar.activation(h_sb[m][:], ps[:], AF.Gelu,
                                         bias=fc1b_sb[:, m:m + 1])

                # fc2 in 3 passes of 2 output chunks so it can start while
                # fc1 is still producing later h chunks
                f2w = [f2s.tile([128, DIM], BF16, tag=f"f2w{kc}", bufs=1,
                                name=f"f2w{kc}") for kc in range(MC1)]
                for kc in range(MC1):
                    nc.sync.dma_start(f2w[kc][:],
                                      fc2_wT[128 * kc:128 * (kc + 1), :])
                for half in range(3):
                    ms = (2 * half, 2 * half + 1)
                    pss = {}
                    for m in ms:
                        pss[m] = fps.tile([128, NT], F32, tag=f"f{m % 2}",
                                          name=f"f{m % 2}")
                    for kc in range(MC1):
                        for m in ms:
                            for lo, n in ((0, N0), (N0, N1)):
                                nc.tensor.matmul(
                                    pss[m][:, lo:lo + n],
                                    f2w[kc][:, 128 * m:128 * (m + 1)],
                                    h_sb[kc][:, lo:lo + n],
                                    start=(kc == 0), stop=False)
                    for m in ms:
                        for lo, n in ((0, N0), (N0, N1)):
                            nc.tensor.matmul(pss[m][:, lo:lo + n],
                                             fc2b_sb[:, 128 * m:128 * (m + 1)],
                                             ones_row_bf[:, lo:lo + n],
                                             start=False, stop=True)
                        x2c = x2s.tile([128, NT], F32, tag="x2c")
                        nc.vector.tensor_tensor(x2c[:], pss[m][:], x1T[m][:],
                                                op=OP.add)
                        nc.sync.dma_start(x2T[128 * m:128 * (m + 1), :],
                                          x2c[:])
            mlp_cm.__exit__(None, None, None)

    nc.finalize()
    return nc


_NC_CACHE = None
LAST_RESULT = None


def _get_nc():
    global _NC_CACHE
    if _NC_CACHE is None:
        _NC_CACHE = build_nc()
    return _NC_CACHE


def kernel(x, global_index_template, global_index_search, norm1_w, norm1_b,
           qkv_w, proj_w, proj_b, norm2_w, norm2_b, fc1_w, fc1_b, fc2_w,
           fc2_b):
    x = np.asarray(x, np.float32)
    shared = {
        "qkv_wT": np.ascontiguousarray(np.asarray(qkv_w, np.float32).T),
        "proj_wT": np.ascontiguousarray(np.asarray(proj_w, np.float32).T),
        "fc1_wT": np.ascontiguousarray(
            np.asarray(fc1_w).T.astype(ml_dtypes.bfloat16)),
        "fc2_wT": np.ascontiguousarray(
            np.asarray(fc2_w).T.astype(ml_dtypes.bfloat16)),
        "n1w": np.ascontiguousarray(np.asarray(norm1_w, np.float32)[:, None]),
        "n1b": np.ascontiguousarray(np.asarray(norm1_b, np.float32)[:, None]),
        "n2w": np.ascontiguousarray(np.asarray(norm2_w, np.float32)[:, None]),
        "n2b": np.ascontiguousarray(np.asarray(norm2_b, np.float32)[:, None]),
        "fc1b": np.ascontiguousarray(
            np.asarray(fc1_b, np.float32).reshape(MC1, 128).T),
        "projb": np.ascontiguousarray(np.asarray(proj_b, np.float32)[None, :]),
        "fc2b": np.ascontiguousarray(
            np.asarray(fc2_b).astype(ml_dtypes.bfloat16)[None, :]),
    }
    in_maps = []
    for b in range(B):
        m = dict(shared)
        xb = x[b]
        m["xT"] = np.ascontiguousarray(xb.T)
        mu = xb.mean(axis=1)
        var = ((xb - mu[:, None]) ** 2).mean(axis=1)
        m["mu1"] = np.ascontiguousarray(mu[None, :].astype(np.float32))
        m["rs1"] = np.ascontiguousarray(
            (1.0 / np.sqrt(var + EPS))[None, :].astype(np.float32))
        in_maps.append(m)

    nc = _get_nc()
    res = bass_utils.run_bass_kernel_spmd(nc, in_maps, core_ids=list(range(B)))
    global LAST_RESULT
    LAST_RESULT = res

    gis = np.asarray(global_index_search)
    x_out = np.empty((B, LT + KEEP, DIM), np.float32)
    keep_index = np.empty((B, KEEP), gis.dtype)
    removed_index = np.empty((B, LS - KEEP), gis.dtype)
    attn_ce = np.empty((B, HEADS, LS, NT), np.float32)
    for b in range(B):
        r = res.results[b]
        d = r["dall"]                        # [H, NT]
        eT = r["ace"]                        # [H, NT(keys), LS(search q)]
        a_ce = eT.transpose(0, 2, 1) / d[:, LT:, None]
        attn_ce[b] = a_ce
        attn_t = np.linalg.norm(a_ce.mean(axis=0), axis=1)
        order = np.argsort(-attn_t, kind="stable")
        topk = order[:KEEP]
        keep_index[b] = gis[b][topk]
        removed_index[b] = gis[b][order[KEEP:]]
        x2 = r["x2T"].T
        x_out[b, :LT] = x2[:LT]
        x_out[b, LT:] = x2[LT:][topk]

    return (x_out, np.asarray(global_index_template), keep_index,
            removed_index, attn_ce)
